# revision 11
# baseline (speedup 1.0000x reference)
"""Two-layer GRU (16->128->128) + FC(128->24) head on 8 Trainium2 NeuronCores.

Strategy: data-parallel over the batch (4096 -> 512 per core); tiny weights
replicated. On each core the hidden state lives transposed in SBUF as
[H=128 partitions, B=512 free]. Per time step, gate pre-activations are
accumulated in PSUM by fp32r matmuls (input-projection + recurrent + biases
folded in), sigmoid/tanh run on the scalar engine with per-partition bias
APs, and the cell update is spread across vector + gpsimd engines.

Self-contained: hardcodes all shapes; host-side prep only reshapes/transposes
numpy arrays (sharding + time-major packing of x, weight transposes).
"""

import numpy as np

import bass_rust
import concourse.bass as bass
import concourse.mybir as mybir
from concourse.tile import TileContext
from concourse.vector_clock import ScopedClock
from concourse.bass_utils import run_bass_kernel_spmd

N_CORES = 8
B_TOT = 4096
L = 128          # sequence length (= 2*1024/16)
D = 16           # per-step input features
DA = 17          # + ones row (bias folding for layer 1)
H = 128          # hidden
G3 = 3 * H       # 384 stacked gates (r, z, n)
BL = B_TOT // N_CORES  # 512 batch per core
NCLS = 24
CHUNK = 16       # time steps of x staged into SBUF per DMA

F32 = mybir.dt.float32
F32R = mybir.dt.float32r
AF = mybir.ActivationFunctionType
OP = mybir.AluOpType


class SplitDrainTileContext(TileContext):
    """Walrus (CoreV3) rejects instructions carrying >2 sync waits; Tile's
    kernel-tail drain accumulates one wait per outstanding engine/DMA-queue
    sem. Split them across a chain of drains (1 wait each)."""

    def _drain_and_barrier(self, tick_clock, wait_clock):
        nc = self.nc
        drain_inst = nc.sync.drain()
        wait_clock.add_sem_waits(
            drain_inst.ins, ScopedClock({None: tick_clock.global_clock})
        )
        si = drain_inst.ins.sync_info
        if si is not None and len(si.on_wait) > 1:
            waits = list(si.on_wait)
            si.on_wait = waits[:1]
            for w in waits[1:]:
                d2 = nc.sync.drain()
                d2.ins.sync_info = bass_rust.SyncInfo(on_wait=[w], on_update=[])
        nc.all_engine_barrier()
        popped = nc._tile_sem_poison_stack.pop()
        assert popped is self._sem_poison
        nc.clear_and_free_semaphores(list(self.sems.allocated().values()))
        nc.all_engine_barrier()


def _split_excess_waits(nc: bass.Bass, max_waits: int = 1) -> None:
    """Walrus (CoreV3 setupSyncWait) accepts at most 2 sem waits per
    instruction; Tile occasionally attaches 3+. Hoist the excess onto
    EventSemaphore instructions inserted right before the offender on the
    same engine (serial waits AND together)."""
    n = 0
    for fn in nc.m.functions:
        for bb in fn.blocks:
            out = []
            dirty = False
            for inst in bb.instructions:
                si = inst.sync_info
                if si is not None and len(si.on_wait) > max_waits:
                    waits = list(si.on_wait)
                    extra = waits[: len(waits) - max_waits]
                    for w in extra:
                        ev = mybir.InstEventSemaphore(
                            name=f"evs-waitsplit-{n}", ins=[], outs=[]
                        )
                        n += 1
                        ev.engine = inst.engine
                        ev.sync_info = bass_rust.SyncInfo(
                            on_wait=[w], on_update=[]
                        )
                        out.append(ev)
                    si.on_wait = waits[len(waits) - max_waits :]
                    dirty = True
                out.append(inst)
            if dirty:
                bb.instructions = out


def build_program(for_sim: bool = False, n_steps: int = L) -> bass.Bass:
    # for_sim: skip the walrus wait-limit workarounds (post-hoc IR mutations
    # that CoreSim's bookkeeping doesn't understand); semantics identical.
    nc = bass.Bass()

    # Per-core DRAM I/O. Matmul operands are declared float32r (same bytes as
    # fp32) so the PE runs them at 1 cycle/row instead of fp32's 4.
    xT_d = nc.declare_dram_parameter("xT", [L, DA, BL], F32R, isOutput=False)
    l1w_d = nc.declare_dram_parameter("l1w", [DA, G3], F32R, isOutput=False)
    hh1_d = nc.declare_dram_parameter("hh1w", [H, G3], F32R, isOutput=False)
    ih2_d = nc.declare_dram_parameter("ih2w", [H, G3], F32R, isOutput=False)
    hh2_d = nc.declare_dram_parameter("hh2w", [H, G3], F32R, isOutput=False)
    bias_d = nc.declare_dram_parameter("bvec", [H, 5], F32, isOutput=False)
    fcw_d = nc.declare_dram_parameter("fcw", [H, NCLS], F32R, isOutput=False)
    fcb_d = nc.declare_dram_parameter("fcb", [NCLS, 1], F32, isOutput=False)
    out_d = nc.declare_dram_parameter("outT", [NCLS, BL], F32, isOutput=True)

    tc_cls = TileContext if for_sim else SplitDrainTileContext
    with tc_cls(nc) as tc:
        with (
            tc.tile_pool(name="singles", bufs=1) as singles,
            tc.tile_pool(name="xchunks", bufs=2) as xpool,
            tc.tile_pool(name="hstate", bufs=2) as hpool,
            tc.tile_pool(name="work", bufs=2) as work,
            tc.tile_pool(name="prz", bufs=1, space="PSUM") as przpool,
            tc.tile_pool(name="pnx", bufs=1, space="PSUM") as pnxpool,
        ):
            # --- constant loads -------------------------------------------
            l1w = singles.tile([DA, G3], F32R, tag="l1w")
            hh1w = singles.tile([H, G3], F32R, tag="hh1w")
            ih2w = singles.tile([H, G3], F32R, tag="ih2w")
            hh2w = singles.tile([H, G3], F32R, tag="hh2w")
            sbias = singles.tile([H, 5], F32, tag="sbias")
            fcw = singles.tile([H, NCLS], F32R, tag="fcw")
            fcb = singles.tile([NCLS, 1], F32, tag="fcb")
            nc.sync.dma_start(out=l1w[:], in_=l1w_d[:])
            nc.sync.dma_start(out=hh1w[:], in_=hh1_d[:])
            nc.sync.dma_start(out=ih2w[:], in_=ih2_d[:])
            nc.sync.dma_start(out=hh2w[:], in_=hh2_d[:])
            nc.sync.dma_start(out=sbias[:], in_=bias_d[:])
            nc.sync.dma_start(out=fcw[:], in_=fcw_d[:])
            nc.sync.dma_start(out=fcb[:], in_=fcb_d[:])

            def cell(tag, h_prev, x_rhs, xw, hw, rz_bias, n_hh_bias, n_ih_bias):
                """One GRU cell step in transposed layout.

                h_prev: [H, BL] f32r tile or None (t=0 => h=0, recurrent
                matmuls skipped). x_rhs: [K, BL] f32r rhs for the input
                projection with lhsT xw [K, G3]; hw: [H, G3] recurrent lhsT.
                rz_bias: None (folded into xw) or (r_bias_ap, z_bias_ap).
                Returns the new [H, BL] f32r hidden tile.
                """
                prz = przpool.tile([H, 2 * BL], F32, tag=f"prz{tag}")
                pn = pnxpool.tile([H, BL], F32, tag=f"pn{tag}")
                px = pnxpool.tile([H, BL], F32, tag=f"px{tag}")
                nc.tensor.matmul(prz[:, 0:BL], xw[:, 0:H], x_rhs,
                                 start=True, stop=h_prev is None)
                nc.tensor.matmul(prz[:, BL:], xw[:, H : 2 * H], x_rhs,
                                 start=True, stop=h_prev is None)
                nc.tensor.matmul(px[:], xw[:, 2 * H :], x_rhs, start=True, stop=True)
                if h_prev is not None:
                    nc.tensor.matmul(prz[:, 0:BL], hw[:, 0:H], h_prev[:],
                                     start=False, stop=True)
                    nc.tensor.matmul(prz[:, BL:], hw[:, H : 2 * H], h_prev[:],
                                     start=False, stop=True)
                    nc.tensor.matmul(pn[:], hw[:, 2 * H :], h_prev[:],
                                     start=True, stop=True)

                if rz_bias is None:
                    rz = work.tile([H, 2 * BL], F32, tag=f"rz{tag}")
                    nc.scalar.activation(rz[:], prz[:], AF.Sigmoid)
                    r, z = rz[:, 0:BL], rz[:, BL:]
                else:
                    rt = work.tile([H, BL], F32, tag=f"r{tag}")
                    nc.scalar.activation(rt[:], prz[:, 0:BL], AF.Sigmoid, bias=rz_bias[0])
                    zt = work.tile([H, BL], F32, tag=f"z{tag}")
                    nc.scalar.activation(zt[:], prz[:, BL:], AF.Sigmoid, bias=rz_bias[1])
                    r, z = rt[:], zt[:]

                t2 = work.tile([H, BL], F32, tag=f"t2{tag}")
                if h_prev is not None:
                    # t2 = (hn + b_hh_n) * r
                    nc.vector.scalar_tensor_tensor(
                        t2[:], pn[:], n_hh_bias, r, op0=OP.add, op1=OP.mult
                    )
                else:
                    nc.vector.tensor_scalar_mul(t2[:], r, n_hh_bias)
                pre = work.tile([H, BL], F32, tag=f"pre{tag}")
                nc.vector.tensor_add(pre[:], t2[:], px[:])
                n = work.tile([H, BL], F32, tag=f"n{tag}")
                if n_ih_bias is None:
                    nc.scalar.activation(n[:], pre[:], AF.Tanh)
                else:
                    nc.scalar.activation(n[:], pre[:], AF.Tanh, bias=n_ih_bias)
                d = work.tile([H, BL], F32, tag=f"d{tag}")
                if h_prev is not None:
                    nc.gpsimd.tensor_sub(d[:], h_prev[:], n[:])
                else:
                    nc.gpsimd.tensor_scalar_mul(d[:], n[:], -1.0)
                e = work.tile([H, BL], F32, tag=f"e{tag}")
                nc.gpsimd.tensor_mul(e[:], z, d[:])
                h_new = hpool.tile([H, BL], F32R, tag=f"h{tag}")
                nc.vector.tensor_add(h_new[:], n[:], e[:])
                return h_new

            h1 = None
            h2 = None
            xc = None
            for t in range(n_steps):
                if t % CHUNK == 0:
                    xc = xpool.tile([DA, CHUNK, BL], F32R, tag="xc")
                    nc.sync.dma_start(
                        out=xc[:], in_=xT_d[t : t + CHUNK].rearrange("t d b -> d t b")
                    )
                xg = xc[:, t % CHUNK, :]
                h1 = cell("1", h1, xg, l1w, hh1w, None, sbias[:, 0:1], None)
                h2 = cell("2", h2, h1[:], ih2w, hh2w,
                          (sbias[:, 1:2], sbias[:, 2:3]), sbias[:, 3:4],
                          sbias[:, 4:5])

            # ---------------- FC head ------------------------------------
            pfc = pnxpool.tile([NCLS, BL], F32, tag="pn1")
            nc.tensor.matmul(pfc[:], fcw[:], h2[:], start=True, stop=True)
            outs = work.tile([NCLS, BL], F32, tag="outs")
            nc.scalar.activation(outs[:], pfc[:], AF.Identity, bias=fcb[:])
            nc.sync.dma_start(out=out_d[:], in_=outs[:])

    if not for_sim:
        _split_excess_waits(nc)
    return nc


def prep_in_maps(inputs: dict) -> list[dict]:
    """Shard + repack the full-problem numpy inputs into per-core in_maps."""
    x = np.ascontiguousarray(np.asarray(inputs["x"], dtype=np.float32))
    w_ih1 = np.asarray(inputs["w_ih1"], np.float32)
    w_hh1 = np.asarray(inputs["w_hh1"], np.float32)
    b_ih1 = np.asarray(inputs["b_ih1"], np.float32)
    b_hh1 = np.asarray(inputs["b_hh1"], np.float32)
    w_ih2 = np.asarray(inputs["w_ih2"], np.float32)
    w_hh2 = np.asarray(inputs["w_hh2"], np.float32)
    b_ih2 = np.asarray(inputs["b_ih2"], np.float32)
    b_hh2 = np.asarray(inputs["b_hh2"], np.float32)
    fc_w = np.asarray(inputs["fc_w"], np.float32)
    fc_b = np.asarray(inputs["fc_b"], np.float32)

    # x: (4096, 2, 1024) -> per-core time-major transposed [L, 17, BL]
    xr = x.reshape(N_CORES, BL, 2, L, D // 2)  # [core, b, ch, t, j]
    xT = np.empty((N_CORES, L, DA, BL), np.float32)
    xT[:, :, 0 : D // 2, :] = xr[:, :, 0].transpose(0, 2, 3, 1)
    xT[:, :, D // 2 : D, :] = xr[:, :, 1].transpose(0, 2, 3, 1)
    xT[:, :, D, :] = 1.0  # ones row: folds layer-1 biases into the matmul

    # layer-1 combined input-proj weights + bias row.
    # r/z columns carry b_ih1+b_hh1; n columns carry b_ih1 only (b_hh1_n must
    # be applied inside r*(hn+b_hh1_n)).
    l1w = np.empty((DA, G3), np.float32)
    l1w[0:D, :] = w_ih1.T
    bias_row = b_ih1.copy()
    bias_row[0 : 2 * H] += b_hh1[0 : 2 * H]
    l1w[D, :] = bias_row

    bvec = np.stack(
        [
            b_hh1[2 * H : 3 * H],                     # col 0: L1 n-gate hh bias
            (b_ih2 + b_hh2)[0:H],                     # col 1: L2 r bias
            (b_ih2 + b_hh2)[H : 2 * H],               # col 2: L2 z bias
            b_hh2[2 * H : 3 * H],                     # col 3: L2 n-gate hh bias
            b_ih2[2 * H : 3 * H],                     # col 4: L2 n-gate ih bias
        ],
        axis=1,
    ).astype(np.float32)

    shared = {
        "l1w": np.ascontiguousarray(l1w),
        "hh1w": np.ascontiguousarray(w_hh1.T),
        "ih2w": np.ascontiguousarray(w_ih2.T),
        "hh2w": np.ascontiguousarray(w_hh2.T),
        "bvec": bvec,
        "fcw": np.ascontiguousarray(fc_w.T),
        "fcb": np.ascontiguousarray(fc_b[:, None]),
    }
    return [{"xT": np.ascontiguousarray(xT[c]), **shared} for c in range(N_CORES)]


def assemble_output(results: list[dict]) -> np.ndarray:
    # per-core outT [24, BL] -> (4096, 24)
    return np.concatenate([r["outT"].T for r in results], axis=0).astype(np.float32)


_NC_CACHE = None


def kernel(**inputs) -> np.ndarray:
    global _NC_CACHE
    if _NC_CACHE is None:
        _NC_CACHE = build_program()
    in_maps = prep_in_maps(inputs)
    res = run_bass_kernel_spmd(_NC_CACHE, in_maps, list(range(N_CORES)))
    return assemble_output(res.results)
